# revision 18
# baseline (speedup 1.0000x reference)
"""Trainium2 Bass kernel for nn_DenseBayesian (dense + hard LWTA grouped argmax mask).

Computes out = x @ W.T + b, then per group of U=4 output units keeps only the
argmax unit (others zeroed). Data-parallel over 8 NeuronCores along the row axis.

Numerics: main product runs in fp16 (xh = fp16(x), wh = fp16(W.T); fp16 x fp16
products are exact in fp32 PSUM). The two dropped cross terms xl@W and x@wl
(xl = x - xh, wl = W.T - wh) are added as fp8-e5m2 DoubleRow matmuls: operand
pairs are pre-scaled by 2^+-6 so both factors sit in e5m2's normal range while
the product scale cancels, and DoubleRow packs the full K=256 contraction into
one pass. Measured end-to-end rel err ~2.9e-3 (winner flips only where the
group's top-2 gap is below the ~1e-4 correction noise).

LWTA mask: Act drains PSUM to an fp32 SBUF copy; DVE computes the exact fp32
group max with one fused reduce; Pool computes the gap d16 = u - max (fp32
subtract, fp16 store - sign-exact: the winner's gap is exactly 0); DVE emits
the masked output with one fused (d16 >= 0) * u multiply. Output travels fp16
and is upcast on host.

All three x operand streams (fp16 hi + two fp8 views) are packed into one
contiguous byte buffer per macro-tile and sliced on-chip via bitcast, so each
macro needs a single input DMA (fewer sync-sequencer configs + descriptors).

Self-contained: hardcodes the problem shapes; needs numpy + ml_dtypes + the
concourse runtime available on the host.
"""
import numpy as np
import ml_dtypes

import concourse.bass as bass
import concourse.mybir as mybir
import concourse.tile as tile
from concourse import bacc
from concourse.bass_utils import run_bass_kernel_spmd

f32 = mybir.dt.float32
f16 = mybir.dt.float16
f8 = mybir.dt.float8e5
u8 = mybir.dt.uint8

N = 262144
DIN = 256
DOUT = 512
U = 4
NCORES = 8
ROWS = N // NCORES          # 32768 rows per core
MACRO = 256                 # rows per macro-tile (2 psum banks of 128 rows)
P = 128
KC = DIN // P               # k chunks (2)
G = DOUT // U               # groups per 512-col half (128)
S8 = 64.0                   # e5m2 operand pre-scale (2^6)
XB = 2 * KC * MACRO         # packed input bytes/partition/macro: fp16 x ...
XBT = XB + KC * MACRO       # ... + fp8 xl8 + fp8 xs8


def build_program(n_macros: int, with_bias: bool, mode: str = "stt"):
    assert mode == "stt"
    nc = bacc.Bacc("TRN2", target_bir_lowering=False)
    rows = n_macros * MACRO
    A = mybir.AluOpType
    AF = mybir.ActivationFunctionType
    DR = mybir.MatmulPerfMode.DoubleRow

    xb_d = nc.dram_tensor("xb", [n_macros, P, XBT + KC * MACRO], u8,
                          kind="ExternalInput")
    wh_d = nc.dram_tensor("wh", [P, KC, DOUT], f16, kind="ExternalInput")
    wq8_d = nc.dram_tensor("wq8", [P, KC, DOUT], f8, kind="ExternalInput")
    wl8_d = nc.dram_tensor("wl8", [P, KC, DOUT], f8, kind="ExternalInput")
    if with_bias:
        bh_d = nc.dram_tensor("bh", [1, DOUT], f16, kind="ExternalInput")
        bl_d = nc.dram_tensor("bl", [1, DOUT], f16, kind="ExternalInput")
    out_d = nc.dram_tensor("out", [rows, DOUT], f16, kind="ExternalOutput")

    with tile.TileContext(nc) as tc:
        with tc.tile_pool(name="wpool", bufs=1) as wpool, \
             tc.tile_pool(name="xpool", bufs=12) as xpool, \
             tc.tile_pool(name="upool", bufs=8) as upool, \
             tc.tile_pool(name="mpool", bufs=8) as mpool, \
             tc.tile_pool(name="kpool", bufs=8) as kpool, \
             tc.tile_pool(name="opool", bufs=8) as opool, \
             tc.tile_pool(name="pspool", bufs=4, space="PSUM") as pspool:

            wh = wpool.tile([P, KC, DOUT], f16)
            nc.sync.dma_start(wh[:], wh_d[:])
            wq8 = wpool.tile([P, KC, DOUT], f8)
            nc.sync.dma_start(wq8[:], wq8_d[:])
            wl8 = wpool.tile([P, KC, DOUT], f8)
            nc.sync.dma_start(wl8[:], wl8_d[:])
            if with_bias:
                bh = wpool.tile([1, DOUT], f16)
                nc.sync.dma_start(bh[:], bh_d[:])
                bl = wpool.tile([1, DOUT], f16)
                nc.sync.dma_start(bl[:], bl_d[:])
                ones = wpool.tile([1, P], f16)
                nc.vector.memset(ones[:], 1.0)

            for mt in range(n_macros):
                xb = xpool.tile([P, XBT + KC * MACRO], u8, tag="xb")
                nc.sync.dma_start(xb[:], xb_d[mt, :, :])
                xh_t = xb[:, 0:XB].bitcast(f16).rearrange(
                    "p (c m) -> p c m", c=KC)
                xl8_t = xb[:, XB:XBT].bitcast(f8).rearrange(
                    "p (c m) -> p c m", c=KC)
                xs8_t = xb[:, XBT:].bitcast(f8).rearrange(
                    "p (c m) -> p c m", c=KC)

                ps = pspool.tile([P, 2 * DOUT], f32)
                for s in range(2):
                    acc = ps[:, s * DOUT:(s + 1) * DOUT]
                    rs = slice(s * P, (s + 1) * P)
                    mms = []
                    if with_bias:
                        mms.append((ones[:, :], bh[:, :], None))
                        mms.append((ones[:, :], bl[:, :], None))
                    for c in range(KC):
                        mms.append((xh_t[:, c, rs], wh[:, c, :], None))
                    mms.append((xl8_t[:, :, rs], wq8[:], DR))
                    last = len(mms) - 1
                    for i, (lhsT, rhs, pm) in enumerate(mms):
                        nc.tensor.matmul(acc, lhsT, rhs,
                                         start=(i == 0), stop=(i == last),
                                         perf_mode=pm)

                # exact fp32 group max straight from PSUM (DVE; runs in
                # parallel with the Act copies)
                m32 = mpool.tile([P, 2 * G], f32)
                psg = ps[:].rearrange("p (g u) -> p g u", u=U)
                nc.vector.tensor_reduce(m32[:], psg, axis=mybir.AxisListType.X,
                                        op=A.max)
                mb = m32[:].unsqueeze(2).broadcast_to([P, 2 * G, U])

                # SBUF copies of the logits (Act is the only cheap psum
                # drain): fp32 for the exact subtract, fp16 for the output
                u32 = upool.tile([P, 2 * DOUT], f32)
                nc.scalar.activation(u32[:], ps[:], AF.Copy)
                ug = u32[:].rearrange("p (g u) -> p g u", u=U)

                # gap d = u - max: fp32 subtract, fp16 store (Pool).
                # Sign-exact: winner's d is exactly 0, losers negative.
                d16 = kpool.tile([P, 2 * G, U], f16)
                nc.gpsimd.tensor_tensor(d16[:], ug, mb, A.subtract)

                # fused (d >= 0) * u -> fp16 masked output (DVE); the value
                # operand reads PSUM directly (no SBUF port pressure, no
                # second Act copy)
                o16 = opool.tile([P, 2 * DOUT], f16)
                nc.vector.scalar_tensor_tensor(
                    o16[:], d16[:].rearrange("p g u -> p (g u)"), 0.0, ps[:],
                    op0=A.is_ge, op1=A.mult)

                dst = out_d[mt * MACRO:(mt + 1) * MACRO, :].rearrange(
                    "(s p) j -> p s j", p=P)
                nc.sync.dma_start(dst, o16[:].rearrange("p (s j) -> p s j", s=2))

    nc.compile()
    return nc


_programs: dict = {}


def _get_program(n_macros: int, with_bias: bool, mode: str = "stt"):
    key = (n_macros, with_bias, mode)
    if key not in _programs:
        _programs[key] = build_program(n_macros, with_bias, mode)
    return _programs[key]


def _q8(a: np.ndarray, scale: float):
    return (a * np.float32(scale)).astype(ml_dtypes.float8_e5m2)


def _tile_x(a: np.ndarray, n_macros: int):
    """[rows, DIN] -> [n_macros, P, KC, MACRO]: k = c*P + p, row = mt*MACRO + r."""
    at = np.ascontiguousarray(a.T)                      # [DIN, rows]
    at = at.reshape(KC, P, n_macros, MACRO)             # [c, p, mt, r]
    return np.ascontiguousarray(at.transpose(2, 1, 0, 3))


def _pack_x(xs: np.ndarray, n_macros: int):
    """[rows, DIN] fp32 -> packed u8 [n_macros, P, 4*KC*MACRO]."""
    hi = xs.astype(np.float16)
    lo = (xs - hi.astype(np.float32)).astype(np.float32)
    xh = _tile_x(hi, n_macros)
    xl8 = _tile_x(_q8(lo, S8), n_macros)
    xs8 = _tile_x(_q8(xs, 1.0 / S8), n_macros)
    nm = n_macros
    return np.concatenate([
        xh.view(np.uint8).reshape(nm, P, -1),
        xl8.view(np.uint8).reshape(nm, P, -1),
        xs8.view(np.uint8).reshape(nm, P, -1)], axis=2)


def _tile_w(a: np.ndarray):
    return np.ascontiguousarray(a.reshape(KC, P, DOUT).transpose(1, 0, 2))


def _pack_w(W: np.ndarray):
    """[DOUT, DIN] fp32 -> (wh f16, wq8 f8, wl8 f8) tiled [P, KC, DOUT] of W.T."""
    wT = np.ascontiguousarray(W.astype(np.float32).T)   # [DIN, DOUT]
    hi = wT.astype(np.float16)
    lo = (wT - hi.astype(np.float32)).astype(np.float32)
    return (_tile_w(hi), _tile_w(_q8(wT, 1.0 / S8)), _tile_w(_q8(lo, S8)))


def _pack_b(b: np.ndarray):
    b32 = b.astype(np.float32).reshape(1, DOUT)
    hi = b32.astype(np.float16)
    lo = (b32 - hi.astype(np.float32)).astype(np.float16)
    return hi, lo


def _in_maps(x, W, b, with_bias, n_macros):
    wh, wq8, wl8 = _pack_w(W)
    maps = []
    for i in range(NCORES):
        xb = _pack_x(x[i * ROWS:(i + 1) * ROWS], n_macros)
        im = {"xb": xb, "wh": wh, "wq8": wq8, "wl8": wl8}
        if with_bias:
            bh, bl = _pack_b(b)
            im["bh"] = bh
            im["bl"] = bl
        maps.append(im)
    return maps


def kernel(x: np.ndarray, W: np.ndarray, b: np.ndarray) -> np.ndarray:
    x = np.asarray(x, dtype=np.float32)
    W = np.asarray(W, dtype=np.float32)
    b = np.asarray(b, dtype=np.float32)
    assert x.shape == (N, DIN) and W.shape == (DOUT, DIN) and b.shape == (DOUT,)

    with_bias = bool(np.any(b))
    n_macros = ROWS // MACRO
    nc = _get_program(n_macros, with_bias)
    maps = _in_maps(x, W, b, with_bias, n_macros)
    res = run_bass_kernel_spmd(nc, maps, list(range(NCORES)))
    return np.concatenate(
        [res.results[i]["out"].astype(np.float32) for i in range(NCORES)], axis=0)


# revision 19
# speedup vs baseline: 1.1197x; 1.1197x over previous
"""Trainium2 Bass kernel for nn_DenseBayesian (dense + hard LWTA grouped argmax mask).

Computes out = x @ W.T + b, then per group of U=4 output units keeps only the
argmax unit (others zeroed). Data-parallel over 8 NeuronCores along the row axis.

Numerics: main product runs in fp16 (xh = fp16(x), wh = fp16(W.T); fp16 x fp16
products are exact in fp32 PSUM). The two dropped cross terms xl@W and x@wl
(xl = x - xh, wl = W.T - wh) are added as fp8-e5m2 DoubleRow matmuls: operand
pairs are pre-scaled by 2^+-6 so both factors sit in e5m2's normal range while
the product scale cancels, and DoubleRow packs the full K=256 contraction into
one pass. Measured end-to-end rel err ~2.9e-3 (winner flips only where the
group's top-2 gap is below the ~1e-4 correction noise).

LWTA mask: Act drains PSUM to an fp32 SBUF copy; DVE computes the exact fp32
group max with one fused reduce; Pool computes the gap d16 = u - max (fp32
subtract, fp16 store - sign-exact: the winner's gap is exactly 0); DVE emits
the masked output with one fused (d16 >= 0) * u multiply. Output travels fp16
and is upcast on host.

All three x operand streams (fp16 hi + two fp8 views) are packed into one
contiguous byte buffer per macro-tile and sliced on-chip via bitcast, so each
macro needs a single input DMA (fewer sync-sequencer configs + descriptors).

Self-contained: hardcodes the problem shapes; needs numpy + ml_dtypes + the
concourse runtime available on the host.
"""
import numpy as np
import ml_dtypes

import concourse.bass as bass
import concourse.mybir as mybir
import concourse.tile as tile
from concourse import bacc
from concourse.bass_utils import run_bass_kernel_spmd

f32 = mybir.dt.float32
f16 = mybir.dt.float16
f8 = mybir.dt.float8e5
u8 = mybir.dt.uint8

N = 262144
DIN = 256
DOUT = 512
U = 4
NCORES = 8
ROWS = N // NCORES          # 32768 rows per core
MACRO = 256                 # rows per macro-tile (2 psum banks of 128 rows)
P = 128
KC = DIN // P               # k chunks (2)
G = DOUT // U               # groups per 512-col half (128)
S8 = 64.0                   # e5m2 operand pre-scale (2^6)
XB = 2 * KC * MACRO         # packed input bytes/partition/macro: fp16 x ...
XBT = XB + KC * MACRO       # ... + fp8 xl8 + fp8 xs8


def build_program(n_macros: int, with_bias: bool, mode: str = "stt"):
    assert mode == "stt"
    nc = bacc.Bacc("TRN2", target_bir_lowering=False)
    rows = n_macros * MACRO
    A = mybir.AluOpType
    AF = mybir.ActivationFunctionType
    DR = mybir.MatmulPerfMode.DoubleRow

    xb_d = nc.dram_tensor("xb", [n_macros, P, XBT + KC * MACRO], u8,
                          kind="ExternalInput")
    wh_d = nc.dram_tensor("wh", [P, KC, DOUT], f16, kind="ExternalInput")
    wq8_d = nc.dram_tensor("wq8", [P, KC, DOUT], f8, kind="ExternalInput")
    wl8_d = nc.dram_tensor("wl8", [P, KC, DOUT], f8, kind="ExternalInput")
    if with_bias:
        bh_d = nc.dram_tensor("bh", [1, DOUT], f16, kind="ExternalInput")
        bl_d = nc.dram_tensor("bl", [1, DOUT], f16, kind="ExternalInput")
    out_d = nc.dram_tensor("out", [rows, DOUT], f16, kind="ExternalOutput")

    with tile.TileContext(nc) as tc:
        with tc.tile_pool(name="wpool", bufs=1) as wpool, \
             tc.tile_pool(name="xpool", bufs=12) as xpool, \
             tc.tile_pool(name="upool", bufs=8) as upool, \
             tc.tile_pool(name="mpool", bufs=8) as mpool, \
             tc.tile_pool(name="kpool", bufs=8) as kpool, \
             tc.tile_pool(name="opool", bufs=8) as opool, \
             tc.tile_pool(name="pspool", bufs=4, space="PSUM") as pspool:

            wh = wpool.tile([P, KC, DOUT], f16)
            nc.sync.dma_start(wh[:], wh_d[:])
            wq8 = wpool.tile([P, KC, DOUT], f8)
            nc.sync.dma_start(wq8[:], wq8_d[:])
            wl8 = wpool.tile([P, KC, DOUT], f8)
            nc.sync.dma_start(wl8[:], wl8_d[:])
            if with_bias:
                bh = wpool.tile([1, DOUT], f16)
                nc.sync.dma_start(bh[:], bh_d[:])
                bl = wpool.tile([1, DOUT], f16)
                nc.sync.dma_start(bl[:], bl_d[:])
                ones = wpool.tile([1, P], f16)
                nc.vector.memset(ones[:], 1.0)

            for mt in range(n_macros):
                xb = xpool.tile([P, XBT + KC * MACRO], u8, tag="xb")
                nc.sync.dma_start(xb[:], xb_d[mt, :, :])
                xh_t = xb[:, 0:XB].bitcast(f16).rearrange(
                    "p (c m) -> p c m", c=KC)
                xl8_t = xb[:, XB:XBT].bitcast(f8).rearrange(
                    "p (c m) -> p c m", c=KC)
                xs8_t = xb[:, XBT:].bitcast(f8).rearrange(
                    "p (c m) -> p c m", c=KC)

                ps = pspool.tile([P, 2 * DOUT], f32)
                for s in range(2):
                    acc = ps[:, s * DOUT:(s + 1) * DOUT]
                    rs = slice(s * P, (s + 1) * P)
                    mms = []
                    if with_bias:
                        mms.append((ones[:, :], bh[:, :], None))
                        mms.append((ones[:, :], bl[:, :], None))
                    for c in range(KC):
                        mms.append((xh_t[:, c, rs], wh[:, c, :], None))
                    mms.append((xl8_t[:, :, rs], wq8[:], DR))
                    mms.append((xs8_t[:, :, rs], wl8[:], DR))
                    last = len(mms) - 1
                    for i, (lhsT, rhs, pm) in enumerate(mms):
                        nc.tensor.matmul(acc, lhsT, rhs,
                                         start=(i == 0), stop=(i == last),
                                         perf_mode=pm)

                # exact fp32 group max straight from PSUM (DVE; runs in
                # parallel with the Act copies)
                m32 = mpool.tile([P, 2 * G], f32)
                psg = ps[:].rearrange("p (g u) -> p g u", u=U)
                nc.vector.tensor_reduce(m32[:], psg, axis=mybir.AxisListType.X,
                                        op=A.max)
                mb = m32[:].unsqueeze(2).broadcast_to([P, 2 * G, U])

                # fp16 copy of the logits for the output values (Act)
                u16 = upool.tile([P, 2 * DOUT], f16)
                nc.scalar.activation(u16[:], ps[:], AF.Copy)

                # winner mask: exact fp32 compare vs broadcast max, straight
                # from PSUM (DVE; one PSUM operand per instruction is legal)
                mask16 = kpool.tile([P, 2 * G, U], f16)
                nc.vector.tensor_tensor(mask16[:], psg, mb, A.is_ge)

                # masked output = mask16 * u16 (DVE 16-bit runs at 2x)
                o16 = opool.tile([P, 2 * DOUT], f16)
                nc.vector.tensor_tensor(
                    o16[:], mask16[:].rearrange("p g u -> p (g u)"), u16[:],
                    A.mult)

                dst = out_d[mt * MACRO:(mt + 1) * MACRO, :].rearrange(
                    "(s p) j -> p s j", p=P)
                nc.sync.dma_start(dst, o16[:].rearrange("p (s j) -> p s j", s=2))

    nc.compile()
    return nc


_programs: dict = {}


def _get_program(n_macros: int, with_bias: bool, mode: str = "stt"):
    key = (n_macros, with_bias, mode)
    if key not in _programs:
        _programs[key] = build_program(n_macros, with_bias, mode)
    return _programs[key]


def _q8(a: np.ndarray, scale: float):
    return (a * np.float32(scale)).astype(ml_dtypes.float8_e5m2)


def _tile_x(a: np.ndarray, n_macros: int):
    """[rows, DIN] -> [n_macros, P, KC, MACRO]: k = c*P + p, row = mt*MACRO + r."""
    at = np.ascontiguousarray(a.T)                      # [DIN, rows]
    at = at.reshape(KC, P, n_macros, MACRO)             # [c, p, mt, r]
    return np.ascontiguousarray(at.transpose(2, 1, 0, 3))


def _pack_x(xs: np.ndarray, n_macros: int):
    """[rows, DIN] fp32 -> packed u8 [n_macros, P, 4*KC*MACRO]."""
    hi = xs.astype(np.float16)
    lo = (xs - hi.astype(np.float32)).astype(np.float32)
    xh = _tile_x(hi, n_macros)
    xl8 = _tile_x(_q8(lo, S8), n_macros)
    xs8 = _tile_x(_q8(xs, 1.0 / S8), n_macros)
    nm = n_macros
    return np.concatenate([
        xh.view(np.uint8).reshape(nm, P, -1),
        xl8.view(np.uint8).reshape(nm, P, -1),
        xs8.view(np.uint8).reshape(nm, P, -1)], axis=2)


def _tile_w(a: np.ndarray):
    return np.ascontiguousarray(a.reshape(KC, P, DOUT).transpose(1, 0, 2))


def _pack_w(W: np.ndarray):
    """[DOUT, DIN] fp32 -> (wh f16, wq8 f8, wl8 f8) tiled [P, KC, DOUT] of W.T."""
    wT = np.ascontiguousarray(W.astype(np.float32).T)   # [DIN, DOUT]
    hi = wT.astype(np.float16)
    lo = (wT - hi.astype(np.float32)).astype(np.float32)
    return (_tile_w(hi), _tile_w(_q8(wT, 1.0 / S8)), _tile_w(_q8(lo, S8)))


def _pack_b(b: np.ndarray):
    b32 = b.astype(np.float32).reshape(1, DOUT)
    hi = b32.astype(np.float16)
    lo = (b32 - hi.astype(np.float32)).astype(np.float16)
    return hi, lo


def _in_maps(x, W, b, with_bias, n_macros):
    wh, wq8, wl8 = _pack_w(W)
    maps = []
    for i in range(NCORES):
        xb = _pack_x(x[i * ROWS:(i + 1) * ROWS], n_macros)
        im = {"xb": xb, "wh": wh, "wq8": wq8, "wl8": wl8}
        if with_bias:
            bh, bl = _pack_b(b)
            im["bh"] = bh
            im["bl"] = bl
        maps.append(im)
    return maps


def kernel(x: np.ndarray, W: np.ndarray, b: np.ndarray) -> np.ndarray:
    x = np.asarray(x, dtype=np.float32)
    W = np.asarray(W, dtype=np.float32)
    b = np.asarray(b, dtype=np.float32)
    assert x.shape == (N, DIN) and W.shape == (DOUT, DIN) and b.shape == (DOUT,)

    with_bias = bool(np.any(b))
    n_macros = ROWS // MACRO
    nc = _get_program(n_macros, with_bias)
    maps = _in_maps(x, W, b, with_bias, n_macros)
    res = run_bass_kernel_spmd(nc, maps, list(range(NCORES)))
    return np.concatenate(
        [res.results[i]["out"].astype(np.float32) for i in range(NCORES)], axis=0)


# revision 20
# speedup vs baseline: 1.1238x; 1.0036x over previous
"""Trainium2 Bass kernel for nn_DenseBayesian (dense + hard LWTA grouped argmax mask).

Computes out = x @ W.T + b, then per group of U=4 output units keeps only the
argmax unit (others zeroed). Data-parallel over 8 NeuronCores along the row axis.

Numerics: main product runs in fp16 (xh = fp16(x), wh = fp16(W.T); fp16 x fp16
products are exact in fp32 PSUM). The two dropped cross terms xl@W and x@wl
(xl = x - xh, wl = W.T - wh) are added as fp8-e5m2 DoubleRow matmuls: operand
pairs are pre-scaled by 2^+-6 so both factors sit in e5m2's normal range while
the product scale cancels, and DoubleRow packs the full K=256 contraction into
one pass. Measured end-to-end rel err ~2.9e-3 (winner flips only where the
group's top-2 gap is below the ~1e-4 correction noise).

LWTA mask: Act drains PSUM to an fp32 SBUF copy; DVE computes the exact fp32
group max with one fused reduce; Pool computes the gap d16 = u - max (fp32
subtract, fp16 store - sign-exact: the winner's gap is exactly 0); DVE emits
the masked output with one fused (d16 >= 0) * u multiply. Output travels fp16
and is upcast on host.

All three x operand streams (fp16 hi + two fp8 views) are packed into one
contiguous byte buffer per macro-tile and sliced on-chip via bitcast, so each
macro needs a single input DMA (fewer sync-sequencer configs + descriptors).

Self-contained: hardcodes the problem shapes; needs numpy + ml_dtypes + the
concourse runtime available on the host.
"""
import numpy as np
import ml_dtypes

import concourse.bass as bass
import concourse.mybir as mybir
import concourse.tile as tile
from concourse import bacc
from concourse.bass_utils import run_bass_kernel_spmd

f32 = mybir.dt.float32
f16 = mybir.dt.float16
f8 = mybir.dt.float8e5
u8 = mybir.dt.uint8

N = 262144
DIN = 256
DOUT = 512
U = 4
NCORES = 8
ROWS = N // NCORES          # 32768 rows per core
MACRO = 256                 # rows per macro-tile (2 psum banks of 128 rows)
P = 128
KC = DIN // P               # k chunks (2)
G = DOUT // U               # groups per 512-col half (128)
S8 = 64.0                   # e5m2 operand pre-scale (2^6)
XB = 2 * KC * MACRO         # packed input bytes/partition/macro: fp16 x ...
XBT = XB + KC * MACRO       # ... + fp8 xl8 + fp8 xs8


def build_program(n_macros: int, with_bias: bool, mode: str = "stt"):
    assert mode == "stt"
    nc = bacc.Bacc("TRN2", target_bir_lowering=False)
    rows = n_macros * MACRO
    A = mybir.AluOpType
    AF = mybir.ActivationFunctionType
    DR = mybir.MatmulPerfMode.DoubleRow

    xb_d = nc.dram_tensor("xb", [n_macros, P, XBT + KC * MACRO], u8,
                          kind="ExternalInput")
    wh_d = nc.dram_tensor("wh", [P, KC, DOUT], f16, kind="ExternalInput")
    wq8_d = nc.dram_tensor("wq8", [P, KC, DOUT], f8, kind="ExternalInput")
    wl8_d = nc.dram_tensor("wl8", [P, KC, DOUT], f8, kind="ExternalInput")
    if with_bias:
        bh_d = nc.dram_tensor("bh", [1, DOUT], f16, kind="ExternalInput")
        bl_d = nc.dram_tensor("bl", [1, DOUT], f16, kind="ExternalInput")
    out_d = nc.dram_tensor("out", [rows, DOUT], f16, kind="ExternalOutput")

    with tile.TileContext(nc) as tc:
        with tc.tile_pool(name="wpool", bufs=1) as wpool, \
             tc.tile_pool(name="xpool", bufs=12) as xpool, \
             tc.tile_pool(name="upool", bufs=8) as upool, \
             tc.tile_pool(name="mpool", bufs=8) as mpool, \
             tc.tile_pool(name="kpool", bufs=8) as kpool, \
             tc.tile_pool(name="opool", bufs=8) as opool, \
             tc.tile_pool(name="pspool", bufs=4, space="PSUM") as pspool:

            wh = wpool.tile([P, KC, DOUT], f16)
            nc.sync.dma_start(wh[:], wh_d[:])
            wq8 = wpool.tile([P, KC, DOUT], f8)
            nc.sync.dma_start(wq8[:], wq8_d[:])
            wl8 = wpool.tile([P, KC, DOUT], f8)
            nc.sync.dma_start(wl8[:], wl8_d[:])
            if with_bias:
                bh = wpool.tile([1, DOUT], f16)
                nc.sync.dma_start(bh[:], bh_d[:])
                bl = wpool.tile([1, DOUT], f16)
                nc.sync.dma_start(bl[:], bl_d[:])
                ones = wpool.tile([1, P], f16)
                nc.vector.memset(ones[:], 1.0)

            for mt in range(n_macros):
                xb = xpool.tile([P, XBT + KC * MACRO], u8, tag="xb")
                nc.sync.dma_start(xb[:], xb_d[mt, :, :])
                xh_t = xb[:, 0:XB].bitcast(f16).rearrange(
                    "p (c m) -> p c m", c=KC)
                xl8_t = xb[:, XB:XBT].bitcast(f8).rearrange(
                    "p (c m) -> p c m", c=KC)
                xs8_t = xb[:, XBT:].bitcast(f8).rearrange(
                    "p (c m) -> p c m", c=KC)

                ps = pspool.tile([P, 2 * DOUT], f32)
                for s in range(2):
                    acc = ps[:, s * DOUT:(s + 1) * DOUT]
                    rs = slice(s * P, (s + 1) * P)
                    mms = []
                    if with_bias:
                        mms.append((ones[:, :], bh[:, :], None))
                        mms.append((ones[:, :], bl[:, :], None))
                    for c in range(KC):
                        mms.append((xh_t[:, c, rs], wh[:, c, :], None))
                    mms.append((xl8_t[:, :, rs], wq8[:], DR))
                    last = len(mms) - 1
                    for i, (lhsT, rhs, pm) in enumerate(mms):
                        nc.tensor.matmul(acc, lhsT, rhs,
                                         start=(i == 0), stop=(i == last),
                                         perf_mode=pm)

                # exact fp32 group max straight from PSUM (DVE; runs in
                # parallel with the Act copies)
                m32 = mpool.tile([P, 2 * G], f32)
                psg = ps[:].rearrange("p (g u) -> p g u", u=U)
                nc.vector.tensor_reduce(m32[:], psg, axis=mybir.AxisListType.X,
                                        op=A.max)
                mb = m32[:].unsqueeze(2).broadcast_to([P, 2 * G, U])

                # fp16 copy of the logits for the output values (Act)
                u16 = upool.tile([P, 2 * DOUT], f16)
                nc.scalar.activation(u16[:], ps[:], AF.Copy)

                # winner mask: exact fp32 compare vs broadcast max, straight
                # from PSUM (DVE; one PSUM operand per instruction is legal)
                mask16 = kpool.tile([P, 2 * G, U], f16)
                nc.vector.tensor_tensor(mask16[:], psg, mb, A.is_ge)

                # masked output = mask16 * u16 (DVE 16-bit runs at 2x)
                o16 = opool.tile([P, 2 * DOUT], f16)
                nc.vector.tensor_tensor(
                    o16[:], mask16[:].rearrange("p g u -> p (g u)"), u16[:],
                    A.mult)

                dst = out_d[mt * MACRO:(mt + 1) * MACRO, :].rearrange(
                    "(s p) j -> p s j", p=P)
                nc.sync.dma_start(dst, o16[:].rearrange("p (s j) -> p s j", s=2))

    nc.compile()
    return nc


_programs: dict = {}


def _get_program(n_macros: int, with_bias: bool, mode: str = "stt"):
    key = (n_macros, with_bias, mode)
    if key not in _programs:
        _programs[key] = build_program(n_macros, with_bias, mode)
    return _programs[key]


def _q8(a: np.ndarray, scale: float):
    return (a * np.float32(scale)).astype(ml_dtypes.float8_e5m2)


def _tile_x(a: np.ndarray, n_macros: int):
    """[rows, DIN] -> [n_macros, P, KC, MACRO]: k = c*P + p, row = mt*MACRO + r."""
    at = np.ascontiguousarray(a.T)                      # [DIN, rows]
    at = at.reshape(KC, P, n_macros, MACRO)             # [c, p, mt, r]
    return np.ascontiguousarray(at.transpose(2, 1, 0, 3))


def _pack_x(xs: np.ndarray, n_macros: int):
    """[rows, DIN] fp32 -> packed u8 [n_macros, P, 4*KC*MACRO]."""
    hi = xs.astype(np.float16)
    lo = (xs - hi.astype(np.float32)).astype(np.float32)
    xh = _tile_x(hi, n_macros)
    xl8 = _tile_x(_q8(lo, S8), n_macros)
    xs8 = _tile_x(_q8(xs, 1.0 / S8), n_macros)
    nm = n_macros
    return np.concatenate([
        xh.view(np.uint8).reshape(nm, P, -1),
        xl8.view(np.uint8).reshape(nm, P, -1),
        xs8.view(np.uint8).reshape(nm, P, -1)], axis=2)


def _tile_w(a: np.ndarray):
    return np.ascontiguousarray(a.reshape(KC, P, DOUT).transpose(1, 0, 2))


def _pack_w(W: np.ndarray):
    """[DOUT, DIN] fp32 -> (wh f16, wq8 f8, wl8 f8) tiled [P, KC, DOUT] of W.T."""
    wT = np.ascontiguousarray(W.astype(np.float32).T)   # [DIN, DOUT]
    hi = wT.astype(np.float16)
    lo = (wT - hi.astype(np.float32)).astype(np.float32)
    return (_tile_w(hi), _tile_w(_q8(wT, 1.0 / S8)), _tile_w(_q8(lo, S8)))


def _pack_b(b: np.ndarray):
    b32 = b.astype(np.float32).reshape(1, DOUT)
    hi = b32.astype(np.float16)
    lo = (b32 - hi.astype(np.float32)).astype(np.float16)
    return hi, lo


def _in_maps(x, W, b, with_bias, n_macros):
    wh, wq8, wl8 = _pack_w(W)
    maps = []
    for i in range(NCORES):
        xb = _pack_x(x[i * ROWS:(i + 1) * ROWS], n_macros)
        im = {"xb": xb, "wh": wh, "wq8": wq8, "wl8": wl8}
        if with_bias:
            bh, bl = _pack_b(b)
            im["bh"] = bh
            im["bl"] = bl
        maps.append(im)
    return maps


def kernel(x: np.ndarray, W: np.ndarray, b: np.ndarray) -> np.ndarray:
    x = np.asarray(x, dtype=np.float32)
    W = np.asarray(W, dtype=np.float32)
    b = np.asarray(b, dtype=np.float32)
    assert x.shape == (N, DIN) and W.shape == (DOUT, DIN) and b.shape == (DOUT,)

    with_bias = bool(np.any(b))
    n_macros = ROWS // MACRO
    nc = _get_program(n_macros, with_bias)
    maps = _in_maps(x, W, b, with_bias, n_macros)
    res = run_bass_kernel_spmd(nc, maps, list(range(NCORES)))
    return np.concatenate(
        [res.results[i]["out"].astype(np.float32) for i in range(NCORES)], axis=0)


# revision 21
# speedup vs baseline: 1.3847x; 1.2321x over previous
"""Trainium2 Bass kernel for nn_DenseBayesian (dense + hard LWTA grouped argmax mask).

Computes out = x @ W.T + b, then per group of U=4 output units keeps only the
argmax unit (others zeroed). Data-parallel over 8 NeuronCores along the row axis.

Numerics: main product runs in fp16 (xh = fp16(x), wh = fp16(W.T); fp16 x fp16
products are exact in fp32 PSUM). The two dropped cross terms xl@W and x@wl
(xl = x - xh, wl = W.T - wh) are added as fp8-e5m2 DoubleRow matmuls: operand
pairs are pre-scaled by 2^+-6 so both factors sit in e5m2's normal range while
the product scale cancels, and DoubleRow packs the full K=256 contraction into
one pass. Measured end-to-end rel err ~2.9e-3 (winner flips only where the
group's top-2 gap is below the ~1e-4 correction noise).

LWTA mask: Act drains PSUM to an fp32 SBUF copy; DVE computes the exact fp32
group max with one fused reduce; Pool computes the gap d16 = u - max (fp32
subtract, fp16 store - sign-exact: the winner's gap is exactly 0); DVE emits
the masked output with one fused (d16 >= 0) * u multiply. Output travels fp16
and is upcast on host.

All three x operand streams (fp16 hi + two fp8 views) are packed into one
contiguous byte buffer per macro-tile and sliced on-chip via bitcast, so each
macro needs a single input DMA (fewer sync-sequencer configs + descriptors).

Self-contained: hardcodes the problem shapes; needs numpy + ml_dtypes + the
concourse runtime available on the host.
"""
import numpy as np
import ml_dtypes

import concourse.bass as bass
import concourse.mybir as mybir
import concourse.tile as tile
from concourse import bacc
from concourse.bass_utils import run_bass_kernel_spmd

f32 = mybir.dt.float32
f16 = mybir.dt.float16
f8 = mybir.dt.float8e5
u8 = mybir.dt.uint8

N = 262144
DIN = 256
DOUT = 512
U = 4
NCORES = 8
ROWS = N // NCORES          # 32768 rows per core
MACRO = 256                 # rows per macro-tile (2 psum banks of 128 rows)
P = 128
KC = DIN // P               # k chunks (2)
G = DOUT // U               # groups per 512-col half (128)
S8 = 64.0                   # e5m2 operand pre-scale (2^6)
XB = 2 * KC * MACRO         # packed input bytes/partition/macro: fp16 x ...
XBT = XB + KC * MACRO       # ... + fp8 xl8 + fp8 xs8


def build_program(n_macros: int, with_bias: bool, mode: str = "stt"):
    assert mode == "stt"
    nc = bacc.Bacc("TRN2", target_bir_lowering=False)
    rows = n_macros * MACRO
    A = mybir.AluOpType
    AF = mybir.ActivationFunctionType
    DR = mybir.MatmulPerfMode.DoubleRow

    xb_d = nc.dram_tensor("xb", [n_macros, P, XBT + KC * MACRO], u8,
                          kind="ExternalInput")
    wh_d = nc.dram_tensor("wh", [P, KC, DOUT], f16, kind="ExternalInput")
    wq8_d = nc.dram_tensor("wq8", [P, KC, DOUT], f8, kind="ExternalInput")
    wl8_d = nc.dram_tensor("wl8", [P, KC, DOUT], f8, kind="ExternalInput")
    if with_bias:
        bh_d = nc.dram_tensor("bh", [1, DOUT], f16, kind="ExternalInput")
        bl_d = nc.dram_tensor("bl", [1, DOUT], f16, kind="ExternalInput")
    out_d = nc.dram_tensor("out", [rows, DOUT], f16, kind="ExternalOutput")

    with tile.TileContext(nc) as tc:
        with tc.tile_pool(name="wpool", bufs=1) as wpool, \
             tc.tile_pool(name="xpool", bufs=12) as xpool, \
             tc.tile_pool(name="upool", bufs=8) as upool, \
             tc.tile_pool(name="mpool", bufs=8) as mpool, \
             tc.tile_pool(name="kpool", bufs=8) as kpool, \
             tc.tile_pool(name="opool", bufs=8) as opool, \
             tc.tile_pool(name="pspool", bufs=4, space="PSUM") as pspool:

            wh = wpool.tile([P, KC, DOUT], f16)
            nc.sync.dma_start(wh[:], wh_d[:])
            wq8 = wpool.tile([P, KC, DOUT], f8)
            nc.sync.dma_start(wq8[:], wq8_d[:])
            wl8 = wpool.tile([P, KC, DOUT], f8)
            nc.sync.dma_start(wl8[:], wl8_d[:])
            if with_bias:
                bh = wpool.tile([1, DOUT], f16)
                nc.sync.dma_start(bh[:], bh_d[:])
                bl = wpool.tile([1, DOUT], f16)
                nc.sync.dma_start(bl[:], bl_d[:])
                ones = wpool.tile([1, P], f16)
                nc.vector.memset(ones[:], 1.0)

            for mt in range(n_macros):
                xb = xpool.tile([P, XBT + KC * MACRO], u8, tag="xb")
                nc.sync.dma_start(xb[:], xb_d[mt, :, :])
                xh_t = xb[:, 0:XB].bitcast(f16).rearrange(
                    "p (c m) -> p c m", c=KC)
                xl8_t = xb[:, XB:XBT].bitcast(f8).rearrange(
                    "p (c m) -> p c m", c=KC)
                xs8_t = xb[:, XBT:].bitcast(f8).rearrange(
                    "p (c m) -> p c m", c=KC)

                ps = pspool.tile([P, 2 * DOUT], f32)
                for s in range(2):
                    acc = ps[:, s * DOUT:(s + 1) * DOUT]
                    rs = slice(s * P, (s + 1) * P)
                    mms = []
                    if with_bias:
                        mms.append((ones[:, :], bh[:, :], None))
                        mms.append((ones[:, :], bl[:, :], None))
                    for c in range(KC):
                        mms.append((xh_t[:, c, rs], wh[:, c, :], None))
                    mms.append((xl8_t[:, :, rs], wq8[:], DR))
                    mms.append((xs8_t[:, :, rs], wl8[:], DR))
                    last = len(mms) - 1
                    for i, (lhsT, rhs, pm) in enumerate(mms):
                        nc.tensor.matmul(acc, lhsT, rhs,
                                         start=(i == 0), stop=(i == last),
                                         perf_mode=pm)

                # exact fp32 group max straight from PSUM (DVE; runs in
                # parallel with the Act copies)
                m32 = mpool.tile([P, 2 * G], f32)
                psg = ps[:].rearrange("p (g u) -> p g u", u=U)
                nc.vector.tensor_reduce(m32[:], psg, axis=mybir.AxisListType.X,
                                        op=A.max)
                mb = m32[:].unsqueeze(2).broadcast_to([P, 2 * G, U])

                # fp16 copy of the logits for the output values (Act)
                u16 = upool.tile([P, 2 * DOUT], f16)
                nc.scalar.activation(u16[:], ps[:], AF.Copy)

                # winner mask: exact fp32 compare vs broadcast max, straight
                # from PSUM (DVE; one PSUM operand per instruction is legal)
                mask16 = kpool.tile([P, 2 * G, U], f16)
                nc.vector.tensor_tensor(mask16[:], psg, mb, A.is_ge)

                # masked output = mask16 * u16 (Pool; DVE is the critical
                # engine and Pool is otherwise idle)
                o16 = opool.tile([P, 2 * DOUT], f16)
                nc.gpsimd.tensor_tensor(
                    o16[:], mask16[:].rearrange("p g u -> p (g u)"), u16[:],
                    A.mult)

                dst = out_d[mt * MACRO:(mt + 1) * MACRO, :].rearrange(
                    "(s p) j -> p s j", p=P)
                nc.sync.dma_start(dst, o16[:].rearrange("p (s j) -> p s j", s=2))

    nc.compile()
    return nc


_programs: dict = {}


def _get_program(n_macros: int, with_bias: bool, mode: str = "stt"):
    key = (n_macros, with_bias, mode)
    if key not in _programs:
        _programs[key] = build_program(n_macros, with_bias, mode)
    return _programs[key]


def _q8(a: np.ndarray, scale: float):
    return (a * np.float32(scale)).astype(ml_dtypes.float8_e5m2)


def _tile_x(a: np.ndarray, n_macros: int):
    """[rows, DIN] -> [n_macros, P, KC, MACRO]: k = c*P + p, row = mt*MACRO + r."""
    at = np.ascontiguousarray(a.T)                      # [DIN, rows]
    at = at.reshape(KC, P, n_macros, MACRO)             # [c, p, mt, r]
    return np.ascontiguousarray(at.transpose(2, 1, 0, 3))


def _pack_x(xs: np.ndarray, n_macros: int):
    """[rows, DIN] fp32 -> packed u8 [n_macros, P, 4*KC*MACRO]."""
    hi = xs.astype(np.float16)
    lo = (xs - hi.astype(np.float32)).astype(np.float32)
    xh = _tile_x(hi, n_macros)
    xl8 = _tile_x(_q8(lo, S8), n_macros)
    xs8 = _tile_x(_q8(xs, 1.0 / S8), n_macros)
    nm = n_macros
    return np.concatenate([
        xh.view(np.uint8).reshape(nm, P, -1),
        xl8.view(np.uint8).reshape(nm, P, -1),
        xs8.view(np.uint8).reshape(nm, P, -1)], axis=2)


def _tile_w(a: np.ndarray):
    return np.ascontiguousarray(a.reshape(KC, P, DOUT).transpose(1, 0, 2))


def _pack_w(W: np.ndarray):
    """[DOUT, DIN] fp32 -> (wh f16, wq8 f8, wl8 f8) tiled [P, KC, DOUT] of W.T."""
    wT = np.ascontiguousarray(W.astype(np.float32).T)   # [DIN, DOUT]
    hi = wT.astype(np.float16)
    lo = (wT - hi.astype(np.float32)).astype(np.float32)
    return (_tile_w(hi), _tile_w(_q8(wT, 1.0 / S8)), _tile_w(_q8(lo, S8)))


def _pack_b(b: np.ndarray):
    b32 = b.astype(np.float32).reshape(1, DOUT)
    hi = b32.astype(np.float16)
    lo = (b32 - hi.astype(np.float32)).astype(np.float16)
    return hi, lo


def _in_maps(x, W, b, with_bias, n_macros):
    wh, wq8, wl8 = _pack_w(W)
    maps = []
    for i in range(NCORES):
        xb = _pack_x(x[i * ROWS:(i + 1) * ROWS], n_macros)
        im = {"xb": xb, "wh": wh, "wq8": wq8, "wl8": wl8}
        if with_bias:
            bh, bl = _pack_b(b)
            im["bh"] = bh
            im["bl"] = bl
        maps.append(im)
    return maps


def kernel(x: np.ndarray, W: np.ndarray, b: np.ndarray) -> np.ndarray:
    x = np.asarray(x, dtype=np.float32)
    W = np.asarray(W, dtype=np.float32)
    b = np.asarray(b, dtype=np.float32)
    assert x.shape == (N, DIN) and W.shape == (DOUT, DIN) and b.shape == (DOUT,)

    with_bias = bool(np.any(b))
    n_macros = ROWS // MACRO
    nc = _get_program(n_macros, with_bias)
    maps = _in_maps(x, W, b, with_bias, n_macros)
    res = run_bass_kernel_spmd(nc, maps, list(range(NCORES)))
    return np.concatenate(
        [res.results[i]["out"].astype(np.float32) for i in range(NCORES)], axis=0)
